# revision 3
# baseline (speedup 1.0000x reference)
"""Trainium2 Bass kernel for the spiking-network scan (nn_Network_75926431858958).

reference semantics per step (f32):
    act  = 0.9*act + x_t
    spk  = act > thr
    freq = 0.95*freq + 0.05*spk
    thr  = where(freq > 0.1, thr + 0.05, thr)
    thr  = where(freq < 0.1, thr / 1.05, thr)
    act  = where(spk, 0, act)
returns (spk_h, act_h, thr_h, freq_h) stacked over T.

Sharding: H axis split into 8 slabs of 128 rows, one per NeuronCore; the
recurrence is elementwise per neuron so there is zero communication.

Per-core layout: per step a [128, 1024] tile (H-rows on partitions, W on the
free axis), split into 2 independent groups of [128, 512] so the
cross-engine recurrence cycle of one group hides under the other group's
work.  Steps are processed in pairs so every DMA moves 1 MiB.

thr/1.05 is computed as thr * fl(1/1.05) via e = 1 + (fl(1/1.05)-1)*c2 which
is exact in f32 for c2 in {0,1}; true divide is not a valid TT ALU op on V3.
"""

import numpy as np

_T, _H, _W = 100, 1024, 1024
_N_CORES = 8
_ROWS = _H // _N_CORES   # 128 rows per core
_GROUPS = 2
_GW = _W // _GROUPS      # 512
_PAIR = 2                # timesteps per DMA batch (1 MiB per transfer)

# f32-exact scalar constants
_R = float(np.float32(1.0) / np.float32(1.05))   # fl(1/1.05)
_KM1 = float(np.float32(_R) - np.float32(1.0))   # r - 1, exact (Sterbenz)

# engine-assignment knobs (rebalance after profiling)
_C105_ON_GPSIMD = True   # c1*0.05 cmp+mul
_C2_ON_GPSIMD = False    # c2 cmp
_STORE_ON_SCALAR = True  # stores on ACT HWDGE ring, loads on SP ring

_nc_cache = None


def _build():
    import concourse.bacc as bacc
    import concourse.mybir as mybir
    from concourse import tile

    F32 = mybir.dt.float32
    Op = mybir.AluOpType
    COPY = mybir.ActivationFunctionType.Copy

    nc = bacc.Bacc(
        "TRN2", target_bir_lowering=False, debug=False, num_devices=_N_CORES
    )
    x = nc.dram_tensor("x", [_T, _ROWS, _W], F32, kind="ExternalInput")
    o_spk = nc.dram_tensor("o_spk", [_T, _ROWS, _W], F32, kind="ExternalOutput")
    o_act = nc.dram_tensor("o_act", [_T, _ROWS, _W], F32, kind="ExternalOutput")
    o_thr = nc.dram_tensor("o_thr", [_T, _ROWS, _W], F32, kind="ExternalOutput")
    o_freq = nc.dram_tensor("o_freq", [_T, _ROWS, _W], F32, kind="ExternalOutput")

    store_eng = nc.scalar if _STORE_ON_SCALAR else nc.sync

    with tile.TileContext(nc) as tc:
        with (
            tc.tile_pool(name="cpool", bufs=1) as cpool,
            tc.tile_pool(name="xpool", bufs=3) as xpool,
            tc.tile_pool(name="spool", bufs=2) as spool,
            tc.tile_pool(name="tpool", bufs=3) as tpool,
        ):
            init_act = cpool.tile([_ROWS, _W], F32, name="init_act")
            init_thr = cpool.tile([_ROWS, _W], F32, name="init_thr")
            init_f95 = cpool.tile([_ROWS, _W], F32, name="init_f95")
            nc.vector.memset(init_act[:], 0.0)
            nc.vector.memset(init_thr[:], 1.0)
            nc.vector.memset(init_f95[:], 0.0)

            prev_act = [init_act[:, g * _GW:(g + 1) * _GW] for g in range(_GROUPS)]
            prev_thr = [init_thr[:, g * _GW:(g + 1) * _GW] for g in range(_GROUPS)]
            prev_f95 = [init_f95[:, g * _GW:(g + 1) * _GW] for g in range(_GROUPS)]

            for p in range(_T // _PAIR):
                t0 = p * _PAIR
                xt = xpool.tile([_ROWS, _PAIR * _W], F32, name=f"xt{p}", tag="xt")
                nc.sync.dma_start(
                    xt[:].rearrange("p (t w) -> p t w", t=_PAIR),
                    x[t0:t0 + _PAIR].rearrange("t p w -> p t w"),
                )
                st_spk = spool.tile(
                    [_ROWS, _PAIR * _W], F32, name=f"sspk{p}", tag="sspk"
                )
                st_act = spool.tile(
                    [_ROWS, _PAIR * _W], F32, name=f"sact{p}", tag="sact"
                )
                st_thr = spool.tile(
                    [_ROWS, _PAIR * _W], F32, name=f"sthr{p}", tag="sthr"
                )
                st_freq = spool.tile(
                    [_ROWS, _PAIR * _W], F32, name=f"sfreq{p}", tag="sfreq"
                )
                for ti in range(_PAIR):
                    t = t0 + ti
                    for g in range(_GROUPS):
                        off = ti * _W + g * _GW
                        sl = slice(off, off + _GW)
                        sfx = f"{p}_{ti}_{g}"

                        # act1 = 0.9*act + x
                        a1 = tpool.tile(
                            [_ROWS, _GW], F32, name=f"a1_{sfx}", tag=f"a1{g}"
                        )
                        nc.vector.scalar_tensor_tensor(
                            a1[:], prev_act[g], 0.9, xt[:, sl],
                            op0=Op.mult, op1=Op.add,
                        )
                        # spk = act1 > thr
                        spk = st_spk[:, sl]
                        nc.vector.tensor_tensor(
                            spk, a1[:], prev_thr[g], op=Op.is_gt
                        )
                        # act = (spk == 0) * act1   (zero reset)
                        nc.vector.scalar_tensor_tensor(
                            st_act[:, sl], spk, 0.0, a1[:],
                            op0=Op.is_equal, op1=Op.mult,
                        )
                        # freq = 0.05*spk + 0.95*freq_prev
                        nc.vector.scalar_tensor_tensor(
                            st_freq[:, sl], spk, 0.05, prev_f95[g],
                            op0=Op.mult, op1=Op.add,
                        )
                        # pre-scale for the next step (skip on last step)
                        if t + 1 < _T:
                            f95 = tpool.tile(
                                [_ROWS, _GW], F32, name=f"f95_{sfx}", tag=f"f95{g}"
                            )
                            nc.scalar.activation(
                                f95[:], st_freq[:, sl], COPY, bias=0.0, scale=0.95
                            )
                            prev_f95[g] = f95[:]
                        # c105 = (freq > 0.1) * 0.05
                        c105 = tpool.tile(
                            [_ROWS, _GW], F32, name=f"c105_{sfx}", tag=f"c105{g}"
                        )
                        eng = nc.gpsimd if _C105_ON_GPSIMD else nc.vector
                        eng.tensor_scalar(
                            c105[:], st_freq[:, sl], 0.1, 0.05, Op.is_gt, Op.mult
                        )
                        # thr_a = thr + c105
                        thra = tpool.tile(
                            [_ROWS, _GW], F32, name=f"thra_{sfx}", tag=f"thra{g}"
                        )
                        nc.gpsimd.tensor_tensor(
                            thra[:], c105[:], prev_thr[g], op=Op.add
                        )
                        # c2 = freq < 0.1 ;  e = 1 + (r-1)*c2
                        c2 = tpool.tile(
                            [_ROWS, _GW], F32, name=f"c2_{sfx}", tag=f"c2{g}"
                        )
                        eng = nc.gpsimd if _C2_ON_GPSIMD else nc.vector
                        eng.tensor_scalar(
                            c2[:], st_freq[:, sl], 0.1, None, Op.is_lt
                        )
                        e = tpool.tile(
                            [_ROWS, _GW], F32, name=f"e_{sfx}", tag=f"e{g}"
                        )
                        nc.scalar.activation(
                            e[:], c2[:], COPY, bias=1.0, scale=_KM1
                        )
                        # thr = thr_a * e
                        nc.gpsimd.tensor_tensor(
                            st_thr[:, sl], thra[:], e[:], op=Op.mult
                        )

                        prev_act[g] = st_act[:, sl]
                        prev_thr[g] = st_thr[:, sl]

                for st, o in (
                    (st_spk, o_spk),
                    (st_act, o_act),
                    (st_thr, o_thr),
                    (st_freq, o_freq),
                ):
                    store_eng.dma_start(
                        o[t0:t0 + _PAIR].rearrange("t p w -> p t w"),
                        st[:].rearrange("p (t w) -> p t w", t=_PAIR),
                    )

    nc.compile()
    return nc


def kernel(x, _trace=False, _trace_kwargs=None):
    global _nc_cache
    from concourse.bass_utils import run_bass_kernel_spmd

    x = np.asarray(x)
    assert x.shape == (_T, _H, _W), x.shape
    x = np.ascontiguousarray(x, dtype=np.float32)

    if _nc_cache is None:
        _nc_cache = _build()

    in_maps = [
        {"x": np.ascontiguousarray(x[:, c * _ROWS:(c + 1) * _ROWS, :])}
        for c in range(_N_CORES)
    ]
    kw = dict(_trace_kwargs or {})
    res = run_bass_kernel_spmd(
        _nc_cache, in_maps, list(range(_N_CORES)), trace=_trace, **kw
    )
    outs = []
    for name in ("o_spk", "o_act", "o_thr", "o_freq"):
        full = np.empty((_T, _H, _W), dtype=np.float32)
        for c in range(_N_CORES):
            full[:, c * _ROWS:(c + 1) * _ROWS, :] = res.results[c][name]
        outs.append(full)
    if _trace:
        return tuple(outs), res
    return tuple(outs)


# revision 4
# speedup vs baseline: 2.1328x; 2.1328x over previous
"""Trainium2 Bass kernel for the spiking-network scan (nn_Network_75926431858958).

reference semantics per step (f32):
    act  = 0.9*act + x_t
    spk  = act > thr
    freq = 0.95*freq + 0.05*spk
    thr  = where(freq > 0.1, thr + 0.05, thr)
    thr  = where(freq < 0.1, thr / 1.05, thr)
    act  = where(spk, 0, act)
returns (spk_h, act_h, thr_h, freq_h) stacked over T.

Sharding: H axis split into 8 slabs of 128 rows, one per NeuronCore; the
recurrence is elementwise per neuron so there is zero communication.

Per-core layout: per step a [128, 1024] tile (H-rows on partitions, W on the
free axis), split into 2 independent groups of [128, 512] so the
cross-engine recurrence cycle of one group hides under the other group's
work.  Steps are processed in pairs so every DMA moves 1 MiB.

Engine split per step/group: DVE runs the six STT/TT ops (EMA, compare,
reset, freq EMA, two threshold compares folded with their scalings via
const tiles), GPSIMD runs the two threshold TT ops, ACT runs the two affine
ops.  Plain TENSOR_SCALAR is avoided entirely: its 2-port DVE perf mode
and the GPSIMD ucode path serialize against each other on the shared SBUF
port (~8 us per op measured).

thr/1.05 is computed as thr * fl(1/1.05) via e = 1 + K*c2 (K = fl(1/1.05)-1,
exact by Sterbenz); true divide is not a valid TT ALU op on V3.
"""

import numpy as np

_T, _H, _W = 100, 1024, 1024
_N_CORES = 8
_ROWS = _H // _N_CORES   # 128 rows per core
_GROUPS = 2
_GW = _W // _GROUPS      # 512
_PAIR = 2                # timesteps per DMA batch (1 MiB per transfer)

# f32-exact scalar constants
_R = float(np.float32(1.0) / np.float32(1.05))   # fl(1/1.05)
_KM1 = float(np.float32(_R) - np.float32(1.0))   # r - 1, exact (Sterbenz)

_STORE_ON_SCALAR = True  # stores on ACT HWDGE ring, loads on SP ring

_nc_cache = None


def _build():
    import concourse.bacc as bacc
    import concourse.mybir as mybir
    from concourse import tile

    F32 = mybir.dt.float32
    Op = mybir.AluOpType
    COPY = mybir.ActivationFunctionType.Copy

    nc = bacc.Bacc(
        "TRN2", target_bir_lowering=False, debug=False, num_devices=_N_CORES
    )
    x = nc.dram_tensor("x", [_T, _ROWS, _W], F32, kind="ExternalInput")
    o_spk = nc.dram_tensor("o_spk", [_T, _ROWS, _W], F32, kind="ExternalOutput")
    o_act = nc.dram_tensor("o_act", [_T, _ROWS, _W], F32, kind="ExternalOutput")
    o_thr = nc.dram_tensor("o_thr", [_T, _ROWS, _W], F32, kind="ExternalOutput")
    o_freq = nc.dram_tensor("o_freq", [_T, _ROWS, _W], F32, kind="ExternalOutput")

    store_eng = nc.scalar if _STORE_ON_SCALAR else nc.sync

    with tile.TileContext(nc) as tc:
        with (
            tc.tile_pool(name="cpool", bufs=1) as cpool,
            tc.tile_pool(name="xpool", bufs=3) as xpool,
            tc.tile_pool(name="spool", bufs=2) as spool,
            tc.tile_pool(name="tpool", bufs=3) as tpool,
        ):
            init_act = cpool.tile([_ROWS, _W], F32, name="init_act")
            init_thr = cpool.tile([_ROWS, _W], F32, name="init_thr")
            init_f95 = cpool.tile([_ROWS, _W], F32, name="init_f95")
            c005 = cpool.tile([_ROWS, _W], F32, name="c005")
            ckm1 = cpool.tile([_ROWS, _W], F32, name="ckm1")
            nc.vector.memset(init_act[:], 0.0)
            nc.vector.memset(init_thr[:], 1.0)
            nc.vector.memset(init_f95[:], 0.0)
            nc.vector.memset(c005[:], 0.05)
            nc.vector.memset(ckm1[:], _KM1)

            prev_act = [init_act[:, g * _GW:(g + 1) * _GW] for g in range(_GROUPS)]
            prev_thr = [init_thr[:, g * _GW:(g + 1) * _GW] for g in range(_GROUPS)]
            prev_f95 = [init_f95[:, g * _GW:(g + 1) * _GW] for g in range(_GROUPS)]
            gc005 = [c005[:, g * _GW:(g + 1) * _GW] for g in range(_GROUPS)]
            gckm1 = [ckm1[:, g * _GW:(g + 1) * _GW] for g in range(_GROUPS)]

            for p in range(_T // _PAIR):
                t0 = p * _PAIR
                xt = xpool.tile([_ROWS, _PAIR * _W], F32, name=f"xt{p}", tag="xt")
                nc.sync.dma_start(
                    xt[:].rearrange("p (t w) -> p t w", t=_PAIR),
                    x[t0:t0 + _PAIR].rearrange("t p w -> p t w"),
                )
                st_spk = spool.tile(
                    [_ROWS, _PAIR * _W], F32, name=f"sspk{p}", tag="sspk"
                )
                st_act = spool.tile(
                    [_ROWS, _PAIR * _W], F32, name=f"sact{p}", tag="sact"
                )
                st_thr = spool.tile(
                    [_ROWS, _PAIR * _W], F32, name=f"sthr{p}", tag="sthr"
                )
                st_freq = spool.tile(
                    [_ROWS, _PAIR * _W], F32, name=f"sfreq{p}", tag="sfreq"
                )
                for ti in range(_PAIR):
                    t = t0 + ti
                    for g in range(_GROUPS):
                        off = ti * _W + g * _GW
                        sl = slice(off, off + _GW)
                        sfx = f"{p}_{ti}_{g}"

                        # act1 = 0.9*act + x
                        a1 = tpool.tile(
                            [_ROWS, _GW], F32, name=f"a1_{sfx}", tag=f"a1{g}"
                        )
                        nc.vector.scalar_tensor_tensor(
                            a1[:], prev_act[g], 0.9, xt[:, sl],
                            op0=Op.mult, op1=Op.add,
                        )
                        # spk = act1 > thr
                        spk = st_spk[:, sl]
                        nc.vector.tensor_tensor(
                            spk, a1[:], prev_thr[g], op=Op.is_gt
                        )
                        # act = (spk == 0) * act1   (zero reset)
                        nc.vector.scalar_tensor_tensor(
                            st_act[:, sl], spk, 0.0, a1[:],
                            op0=Op.is_equal, op1=Op.mult,
                        )
                        # freq = 0.05*spk + 0.95*freq_prev
                        nc.vector.scalar_tensor_tensor(
                            st_freq[:, sl], spk, 0.05, prev_f95[g],
                            op0=Op.mult, op1=Op.add,
                        )
                        # pre-scale for the next step (skip on last step)
                        if t + 1 < _T:
                            f95 = tpool.tile(
                                [_ROWS, _GW], F32, name=f"f95_{sfx}", tag=f"f95{g}"
                            )
                            nc.scalar.activation(
                                f95[:], st_freq[:, sl], COPY, bias=0.0, scale=0.95
                            )
                            prev_f95[g] = f95[:]
                        # c105 = (freq > 0.1) * 0.05
                        c105 = tpool.tile(
                            [_ROWS, _GW], F32, name=f"c105_{sfx}", tag=f"c105{g}"
                        )
                        nc.vector.scalar_tensor_tensor(
                            c105[:], st_freq[:, sl], 0.1, gc005[g],
                            op0=Op.is_gt, op1=Op.mult,
                        )
                        # thr_a = thr + c105
                        thra = tpool.tile(
                            [_ROWS, _GW], F32, name=f"thra_{sfx}", tag=f"thra{g}"
                        )
                        nc.gpsimd.tensor_tensor(
                            thra[:], c105[:], prev_thr[g], op=Op.add
                        )
                        # kc2 = K*(freq < 0.1) ;  e = 1 + kc2
                        kc2 = tpool.tile(
                            [_ROWS, _GW], F32, name=f"kc2_{sfx}", tag=f"kc2{g}"
                        )
                        nc.vector.scalar_tensor_tensor(
                            kc2[:], st_freq[:, sl], 0.1, gckm1[g],
                            op0=Op.is_lt, op1=Op.mult,
                        )
                        e = tpool.tile(
                            [_ROWS, _GW], F32, name=f"e_{sfx}", tag=f"e{g}"
                        )
                        nc.scalar.activation(
                            e[:], kc2[:], COPY, bias=1.0, scale=1.0
                        )
                        # thr = thr_a * e
                        nc.gpsimd.tensor_tensor(
                            st_thr[:, sl], thra[:], e[:], op=Op.mult
                        )

                        prev_act[g] = st_act[:, sl]
                        prev_thr[g] = st_thr[:, sl]

                for st, o in (
                    (st_spk, o_spk),
                    (st_act, o_act),
                    (st_thr, o_thr),
                    (st_freq, o_freq),
                ):
                    store_eng.dma_start(
                        o[t0:t0 + _PAIR].rearrange("t p w -> p t w"),
                        st[:].rearrange("p (t w) -> p t w", t=_PAIR),
                    )

    nc.compile()
    return nc


def kernel(x, _trace=False, _trace_kwargs=None):
    global _nc_cache
    from concourse.bass_utils import run_bass_kernel_spmd

    x = np.asarray(x)
    assert x.shape == (_T, _H, _W), x.shape
    x = np.ascontiguousarray(x, dtype=np.float32)

    if _nc_cache is None:
        _nc_cache = _build()

    in_maps = [
        {"x": np.ascontiguousarray(x[:, c * _ROWS:(c + 1) * _ROWS, :])}
        for c in range(_N_CORES)
    ]
    kw = dict(_trace_kwargs or {})
    res = run_bass_kernel_spmd(
        _nc_cache, in_maps, list(range(_N_CORES)), trace=_trace, **kw
    )
    outs = []
    for name in ("o_spk", "o_act", "o_thr", "o_freq"):
        full = np.empty((_T, _H, _W), dtype=np.float32)
        for c in range(_N_CORES):
            full[:, c * _ROWS:(c + 1) * _ROWS, :] = res.results[c][name]
        outs.append(full)
    if _trace:
        return tuple(outs), res
    return tuple(outs)


# revision 11
# speedup vs baseline: 2.3306x; 1.0927x over previous
"""Trainium2 Bass kernel for the spiking-network scan (nn_Network_75926431858958).

reference semantics per step (f32):
    act  = 0.9*act + x_t
    spk  = act > thr
    freq = 0.95*freq + 0.05*spk
    thr  = where(freq > 0.1, thr + 0.05, thr)
    thr  = where(freq < 0.1, thr / 1.05, thr)
    act  = where(spk, 0, act)
returns (spk_h, act_h, thr_h, freq_h) stacked over T.

Sharding: H axis split into 8 slabs of 128 rows, one per NeuronCore; the
recurrence is elementwise per neuron so there is zero communication.

Per-core layout: per step a [128, 1024] tile (H-rows on partitions, W on the
free axis), split into 2 independent groups of [128, 512] so the
cross-engine recurrence cycle of one group hides under the other group's
work.  Steps are processed in pairs so every DMA moves 1 MiB.

Engine split per step/group: DVE runs the six STT/TT ops (EMA, compare,
reset, freq EMA, two threshold compares folded with their scalings via
const tiles), GPSIMD runs the two threshold TT ops, ACT runs the two affine
ops.  Plain TENSOR_SCALAR is avoided entirely: its 2-port DVE perf mode
and the GPSIMD ucode path serialize against each other on the shared SBUF
port (~8 us per op measured).

thr/1.05 is computed as thr * fl(1/1.05) via e = 1 + K*c2 (K = fl(1/1.05)-1,
exact by Sterbenz); true divide is not a valid TT ALU op on V3.
"""

import numpy as np

_T, _H, _W = 100, 1024, 1024
_N_CORES = 8
_ROWS = _H // _N_CORES   # 128 rows per core
_GROUPS = 2
_GW = _W // _GROUPS      # 512
_PAIR = 2                # timesteps per DMA batch (1 MiB per transfer)

# f32-exact scalar constants
_R = float(np.float32(1.0) / np.float32(1.05))   # fl(1/1.05)
_KM1 = float(np.float32(_R) - np.float32(1.0))   # r - 1, exact (Sterbenz)

_STORE_ON_SCALAR = False  # stores on ACT HWDGE ring vs SP ring
_CMP_ON_ACT = True        # threshold compares via Sign/Relu on ACT engine
_F95_ON_ACT = True        # freq pre-scale on ACT vs DVE
_THR_ON_GPSIMD = True     # thr TT ops on GPSIMD vs DVE

_nc_cache = None


def _build():
    import concourse.bacc as bacc
    import concourse.mybir as mybir
    from concourse import tile

    F32 = mybir.dt.float32
    Op = mybir.AluOpType
    COPY = mybir.ActivationFunctionType.Copy
    SIGN = mybir.ActivationFunctionType.Sign
    RELU = mybir.ActivationFunctionType.Relu

    nc = bacc.Bacc(
        "TRN2", target_bir_lowering=False, debug=False, num_devices=_N_CORES
    )
    x = nc.dram_tensor("x", [_T, _ROWS, _W], F32, kind="ExternalInput")
    o_spk = nc.dram_tensor("o_spk", [_T, _ROWS, _W], F32, kind="ExternalOutput")
    o_act = nc.dram_tensor("o_act", [_T, _ROWS, _W], F32, kind="ExternalOutput")
    o_thr = nc.dram_tensor("o_thr", [_T, _ROWS, _W], F32, kind="ExternalOutput")
    o_freq = nc.dram_tensor("o_freq", [_T, _ROWS, _W], F32, kind="ExternalOutput")

    store_eng = nc.scalar if _STORE_ON_SCALAR else nc.sync

    with tile.TileContext(nc) as tc:
        with (
            tc.tile_pool(name="cpool", bufs=1) as cpool,
            tc.tile_pool(name="xpool", bufs=3) as xpool,
            tc.tile_pool(name="spool", bufs=2) as spool,
            tc.tile_pool(name="tpool", bufs=3) as tpool,
        ):
            init_act = cpool.tile([_ROWS, _W], F32, name="init_act")
            init_thr = cpool.tile([_ROWS, _W], F32, name="init_thr")
            init_f95 = cpool.tile([_ROWS, _W], F32, name="init_f95")
            c005 = cpool.tile([_ROWS, _W], F32, name="c005")
            ckm1 = cpool.tile([_ROWS, _W], F32, name="ckm1")
            czero = cpool.tile([_ROWS, _W], F32, name="czero")
            bm01 = cpool.tile([_ROWS, 1], F32, name="bm01")
            nc.vector.memset(bm01[:], -0.1)
            nc.vector.memset(init_act[:], 0.0)
            nc.vector.memset(init_thr[:], 1.0)
            nc.vector.memset(init_f95[:], 0.0)
            nc.vector.memset(c005[:], 0.05)
            nc.vector.memset(ckm1[:], _KM1)
            nc.vector.memset(czero[:], 0.0)

            prev_act = [init_act[:, g * _GW:(g + 1) * _GW] for g in range(_GROUPS)]
            prev_thr = [init_thr[:, g * _GW:(g + 1) * _GW] for g in range(_GROUPS)]
            prev_f95 = [init_f95[:, g * _GW:(g + 1) * _GW] for g in range(_GROUPS)]
            gc005 = [c005[:, g * _GW:(g + 1) * _GW] for g in range(_GROUPS)]
            gckm1 = [ckm1[:, g * _GW:(g + 1) * _GW] for g in range(_GROUPS)]
            gzero = [czero[:, g * _GW:(g + 1) * _GW] for g in range(_GROUPS)]

            for p in range(_T // _PAIR):
                t0 = p * _PAIR
                xt = xpool.tile([_ROWS, _PAIR * _W], F32, name=f"xt{p}", tag="xt")
                nc.sync.dma_start(
                    xt[:].rearrange("p (t w) -> p t w", t=_PAIR),
                    x[t0:t0 + _PAIR].rearrange("t p w -> p t w"),
                )
                st_spk = spool.tile(
                    [_ROWS, _PAIR * _W], F32, name=f"sspk{p}", tag="sspk"
                )
                st_act = spool.tile(
                    [_ROWS, _PAIR * _W], F32, name=f"sact{p}", tag="sact"
                )
                st_thr = spool.tile(
                    [_ROWS, _PAIR * _W], F32, name=f"sthr{p}", tag="sthr"
                )
                st_freq = spool.tile(
                    [_ROWS, _PAIR * _W], F32, name=f"sfreq{p}", tag="sfreq"
                )
                for ti in range(_PAIR):
                    t = t0 + ti
                    for g in range(_GROUPS):
                        off = ti * _W + g * _GW
                        sl = slice(off, off + _GW)
                        sfx = f"{p}_{ti}_{g}"

                        # act1 = 0.9*act + x
                        a1 = tpool.tile(
                            [_ROWS, _GW], F32, name=f"a1_{sfx}", tag=f"a1{g}"
                        )
                        nc.vector.scalar_tensor_tensor(
                            a1[:], prev_act[g], 0.9, xt[:, sl],
                            op0=Op.mult, op1=Op.add,
                        )
                        # spk = act1 > thr
                        spk = st_spk[:, sl]
                        nc.vector.tensor_tensor(
                            spk, a1[:], prev_thr[g], op=Op.is_gt
                        )
                        # act = (spk == 0) * act1   (zero reset)
                        nc.vector.scalar_tensor_tensor(
                            st_act[:, sl], spk, 0.0, a1[:],
                            op0=Op.is_equal, op1=Op.mult,
                        )
                        # freq = 0.05*spk + 0.95*freq_prev
                        nc.vector.scalar_tensor_tensor(
                            st_freq[:, sl], spk, 0.05, prev_f95[g],
                            op0=Op.mult, op1=Op.add,
                        )
                        # pre-scale for the next step (skip on last step)
                        if t + 1 < _T:
                            f95 = tpool.tile(
                                [_ROWS, _GW], F32, name=f"f95_{sfx}", tag=f"f95{g}"
                            )
                            if _F95_ON_ACT:
                                nc.scalar.activation(
                                    f95[:], st_freq[:, sl], COPY,
                                    bias=0.0, scale=0.95,
                                )
                            else:
                                nc.vector.scalar_tensor_tensor(
                                    f95[:], st_freq[:, sl], 0.95, gzero[g],
                                    op0=Op.mult, op1=Op.add,
                                )
                            prev_f95[g] = f95[:]
                        c105 = tpool.tile(
                            [_ROWS, _GW], F32, name=f"c105_{sfx}", tag=f"c105{g}"
                        )
                        e = tpool.tile(
                            [_ROWS, _GW], F32, name=f"e_{sfx}", tag=f"e{g}"
                        )
                        if _CMP_ON_ACT:
                            # sgn = sign(freq - 0.1) in {-1, 0, 1}
                            sgn = tpool.tile(
                                [_ROWS, _GW], F32, name=f"sgn_{sfx}", tag=f"sgn{g}"
                            )
                            nc.scalar.activation(
                                sgn[:], st_freq[:, sl], SIGN, bias=bm01[:], scale=1.0
                            )
                            # c105 = relu(0.05*sgn) = 0.05 iff freq > 0.1
                            nc.scalar.activation(
                                c105[:], sgn[:], RELU, bias=0.0, scale=0.05
                            )
                            # nk = relu(K*sgn) = -K iff freq < 0.1   (K < 0)
                            nk = tpool.tile(
                                [_ROWS, _GW], F32, name=f"nk_{sfx}", tag=f"nk{g}"
                            )
                            nc.scalar.activation(
                                nk[:], sgn[:], RELU, bias=0.0, scale=_KM1
                            )
                            # e = 1 - nk in {1, 1+K = fl(1/1.05)}
                            nc.scalar.activation(
                                e[:], nk[:], COPY, bias=1.0, scale=-1.0
                            )
                        else:
                            # c105 = (freq > 0.1) * 0.05
                            nc.vector.scalar_tensor_tensor(
                                c105[:], st_freq[:, sl], 0.1, gc005[g],
                                op0=Op.is_gt, op1=Op.mult,
                            )
                            # kc2 = K*(freq < 0.1) ;  e = 1 + kc2
                            kc2 = tpool.tile(
                                [_ROWS, _GW], F32, name=f"kc2_{sfx}", tag=f"kc2{g}"
                            )
                            nc.vector.scalar_tensor_tensor(
                                kc2[:], st_freq[:, sl], 0.1, gckm1[g],
                                op0=Op.is_lt, op1=Op.mult,
                            )
                            nc.scalar.activation(
                                e[:], kc2[:], COPY, bias=1.0, scale=1.0
                            )
                        # thr = (thr + c105) * e
                        thr_eng = nc.gpsimd if _THR_ON_GPSIMD else nc.vector
                        thra = tpool.tile(
                            [_ROWS, _GW], F32, name=f"thra_{sfx}", tag=f"thra{g}"
                        )
                        thr_eng.tensor_tensor(
                            thra[:], c105[:], prev_thr[g], op=Op.add
                        )
                        thr_eng.tensor_tensor(
                            st_thr[:, sl], thra[:], e[:], op=Op.mult
                        )

                        prev_act[g] = st_act[:, sl]
                        prev_thr[g] = st_thr[:, sl]

                for st, o in (
                    (st_spk, o_spk),
                    (st_act, o_act),
                    (st_thr, o_thr),
                    (st_freq, o_freq),
                ):
                    store_eng.dma_start(
                        o[t0:t0 + _PAIR].rearrange("t p w -> p t w"),
                        st[:].rearrange("p (t w) -> p t w", t=_PAIR),
                    )

    nc.compile()
    return nc


def kernel(x, _trace=False, _trace_kwargs=None):
    global _nc_cache
    from concourse.bass_utils import run_bass_kernel_spmd

    x = np.asarray(x)
    assert x.shape == (_T, _H, _W), x.shape
    x = np.ascontiguousarray(x, dtype=np.float32)

    if _nc_cache is None:
        _nc_cache = _build()

    in_maps = [
        {"x": np.ascontiguousarray(x[:, c * _ROWS:(c + 1) * _ROWS, :])}
        for c in range(_N_CORES)
    ]
    kw = dict(_trace_kwargs or {})
    res = run_bass_kernel_spmd(
        _nc_cache, in_maps, list(range(_N_CORES)), trace=_trace, **kw
    )
    outs = []
    for name in ("o_spk", "o_act", "o_thr", "o_freq"):
        full = np.empty((_T, _H, _W), dtype=np.float32)
        for c in range(_N_CORES):
            full[:, c * _ROWS:(c + 1) * _ROWS, :] = res.results[c][name]
        outs.append(full)
    if _trace:
        return tuple(outs), res
    return tuple(outs)
